# revision 43
# baseline (speedup 1.0000x reference)
"""Multi-head attention (S=2048, B=2, E=1024, H=16, D=64) on 8 Trainium2 cores.

Sharding: batch*heads head-parallel. Core c owns heads {2c, 2c+1} for both
batch elements (4 of the 32 (b,h) attention pairs). Host-side prep: slice/
scale/cast in_proj weights per core, cast x to bf16 laid out token-chunk-major
as [B, NQC, 128, KT, 512] so projection of q-chunk t needs only 1/8th of x;
the output is written head-major [B, 2, 64, NQC, 512] and reassembled to
[S, B, E] on the host (pure layout, no FLOPs). Per core:
  1. ALL inputs stream on the sync DMA queue as ONE need-ordered sequence
     (id64, bias, weights, b0 t0..t3, b1 t0..t3): desc-gens run at kernel
     start on the idle sync sequencer, the scalar queue stays exclusive to
     Exp dispatch, and (b0,t0) projection starts as soon as chunk 0 lands
     (~16us). A PE warm-up burst on id64 beats the HAM cold clock. All other
     projection work is pumped into the attention loops in emission-deadline
     order (generator yields one PE op per pump slot; rates 6/2/2... per
     attend keep every consumer emitted before its producer-read).
  2. q,k,v projected transposed ([col, tok], col = 2 heads x 64) with fp32
     PSUM accumulation over 8 E-tiles; bias added during DVE evacuation
     (q/bias pre-scaled by D^-0.5); head chunks borrow the idle sc PSUM pool.
     v is PE-transposed per 128-kpos tile into va [128, kt, 160] =
     [v_h0 |1| pad | v_h1@80 |1| pad] (ones from one memset = softmax
     denominators; 16-aligned offsets).
  3. Attention per (b, q-chunk of 512): per kpos tile both heads' score MMs
     fill one [128,1024] PSUM tile concurrently (disjoint PE row groups);
     one ScalarE Exp evacuates to SBUF bf16 (the engine floor: 128x ~1.06us).
     attT accumulation (va.T @ exp, row 64 = sum(exp)) is DEFERRED BY FOUR
     kt iterations, split around the score pair [att_h0(kt-4), sc(kt),
     att_h1(kt-4)], and each attend's last 4 att pairs + normalize are
     carried as thunks into the NEXT attend's kt loop (drained 3/kt): the
     Exp stream crosses attend boundaries gaplessly (b1 phase measures
     back-to-back 1004ns Exps).
  4. Normalize without PE transposes: DVE copy of the PSUM denominator row
     (reciprocal_approx_fast is bitwise and misreads PSUM on HW!) -> DVE
     reciprocal_approx_fast -> gpsimd partition_broadcast to [64,512] ->
     DVE multiply (att PSUM x rb SBUF) -> DMA out on sync.
Measured on trn2: 207.6us HW exec; rel err 3.8e-3.
Previous-session baseline (output PE-transposed + per-qchunk normalize
matmul-transposes): 225.8us.
"""

import numpy as np
import ml_dtypes

S, B, E = 2048, 2, 1024
H, D = 16, 64
SCALING = D ** -0.5
NCORES = 8
SB = S * B
HPC = H // NCORES     # 2 heads per core
KT = E // 128         # 8 contraction tiles over E
QCHUNK = 512
NQC = S // QCHUNK     # 4 q/token chunks
NKT = S // 128        # 16 kpos tiles
# va cols: [v_h0(64) | 1 | pad | v_h1(64)@80 | 1 | pad] — head offsets are
# 16-element aligned (XBAR transpose-DMA writes 16-element chunks; unaligned
# destinations corrupt neighbors on HW), and VN itself is 16-aligned so every
# kt slot stays aligned.
VN = 160
VOFF = 80             # head 1 column offset

_BF16 = ml_dtypes.bfloat16
_BUILT = {}


def _build_bass():
    import concourse.bacc as bacc
    import concourse.mybir as mybir
    import concourse.tile as tile
    from contextlib import ExitStack

    f32 = mybir.dt.float32
    bf = mybir.dt.bfloat16

    nc = bacc.Bacc(None, target_bir_lowering=False, debug=False)

    xch_in = nc.dram_tensor("xch", [B, NQC, 128, KT, QCHUNK], bf, kind="ExternalInput")
    wqkv_in = nc.dram_tensor("wqkv", [128, KT, 384], bf, kind="ExternalInput")
    bqkv_in = nc.dram_tensor("bqkv", [384, 1], f32, kind="ExternalInput")
    id64_in = nc.dram_tensor("id64", [128, 64], bf, kind="ExternalInput")
    out_d = nc.dram_tensor("out", [B, HPC, D, NQC, QCHUNK], f32, kind="ExternalOutput")

    with tile.TileContext(nc) as tc, ExitStack() as ctx:
        const = ctx.enter_context(tc.tile_pool(name="const", bufs=1))
        res = ctx.enter_context(tc.tile_pool(name="res", bufs=1))
        expp = ctx.enter_context(tc.tile_pool(name="expp", bufs=8))
        rcp = ctx.enter_context(tc.tile_pool(name="rcp", bufs=4))
        rbp = ctx.enter_context(tc.tile_pool(name="rbp", bufs=4))
        osp = ctx.enter_context(tc.tile_pool(name="osp", bufs=4))
        ps_sc = ctx.enter_context(tc.tile_pool(name="ps_sc", bufs=2, space="PSUM"))
        ps_att = ctx.enter_context(tc.tile_pool(name="ps_att", bufs=2, space="PSUM"))
        ps_pj = ctx.enter_context(tc.tile_pool(name="ps_pj", bufs=1, space="PSUM"))
        ps_vt = ctx.enter_context(tc.tile_pool(name="ps_vt", bufs=1, space="PSUM"))

        # ---- ALL inputs on the sync queue as ONE need-ordered stream: the
        # data lands in exactly this order at full aggregate DMA bandwidth,
        # the desc-gens all run at kernel start on the (otherwise idle) sync
        # sequencer, and the scalar queue stays exclusive to Exp dispatch.
        id64 = const.tile([128, D], bf, tag="id64")
        nc.sync.dma_start(out=id64[:], in_=id64_in[:])
        bqkv_sb = const.tile([128, 3], f32, tag="bqkv")
        nc.sync.dma_start(
            out=bqkv_sb[:], in_=bqkv_in.rearrange("(c p) o -> p (c o)", p=128)
        )
        wq_all = const.tile([128, KT, 384], bf, tag="wqkv")
        nc.sync.dma_start(out=wq_all[:], in_=wqkv_in[:])

        xch = [
            [res.tile([128, KT, QCHUNK], bf, tag=f"xch{b}_{t}", name=f"xch{b}_{t}")
             for t in range(NQC)]
            for b in range(B)
        ]
        for b in range(B):
            for t in range(NQC):
                nc.sync.dma_start(out=xch[b][t][:], in_=xch_in[b, t])

        qT = [res.tile([128, S], bf, tag=f"qT{b}", name=f"qTt{b}") for b in range(B)]
        kT = [res.tile([128, S], bf, tag=f"kT{b}", name=f"kTt{b}") for b in range(B)]
        vT = [res.tile([128, S], bf, tag=f"vT{b}", name=f"vTt{b}") for b in range(B)]
        va = [res.tile([128, NKT, VN], bf, tag=f"va{b}", name=f"vat{b}") for b in range(B)]
        for b in range(B):
            nc.vector.memset(va[b][:], 1.0)  # ones cols survive at 64, 129

        # PE warm-up burst on id64 (lands in ~1us, long before wqkv/x) while
        # the x stream is still in flight (HAM cold clock)
        warm = ps_pj.tile([128, QCHUNK], f32, tag="pj", name="warm")
        for _ in range(30):
            nc.tensor.matmul(
                warm[0:D, 0:D], lhsT=id64[0:D, :], rhs=id64[0:D, :],
                start=True, stop=True,
            )

        def proj_chunk(b, t, which, pumped, vt=True, head=False):
            # out[col, tok] for col-chunk `which` (0=q, 1=k, 2=v), tokens of
            # chunk t. Generator when pumped: yields after each PE op.
            # Head chunks borrow the (still idle) sc pool so the three proj
            # groups don't serialize on the single ps_pj slot.
            dst = (qT[b], kT[b], vT[b])[which]
            if head:
                ps = ps_sc.tile([128, 1024], f32, tag="sc", name="hproj")[:, 0:QCHUNK]
            else:
                ps = ps_pj.tile([128, QCHUNK], f32, tag="pj", name="projps")
            for k in range(KT):
                nc.tensor.matmul(
                    ps[:],
                    lhsT=wq_all[:, k, which * 128:(which + 1) * 128],
                    rhs=xch[b][t][:, k, :],
                    start=(k == 0),
                    stop=(k == KT - 1),
                )
                if k < KT - 1:
                    if pumped:
                        yield
                    continue
                nc.vector.tensor_scalar_add(
                    out=dst[:, t * QCHUNK:(t + 1) * QCHUNK],
                    in0=ps[:],
                    scalar1=bqkv_sb[:, which:which + 1],
                )
                if pumped:
                    yield
            if which == 2 and vt:
                yield from vt_chunk(b, t, pumped)

        def vt_chunk(b, t, pumped=True):
            # PE transpose + DVE evacuation into va. (Both XBAR-transpose-DMA
            # variants measured SLOWER: ~1.2us desc-gen per op serializes on
            # whichever sequencer issues it — even for b1's slack-rich va.)
            for kt in range(4 * t, 4 * t + 4):
                for h in range(HPC):
                    pst = ps_vt.tile([128, D], bf, tag="vt", name="vtps")
                    nc.tensor.transpose(
                        pst[:],
                        in_=vT[b][h * 64:(h + 1) * 64, kt * 128:(kt + 1) * 128],
                        identity=id64[h * 64:(h + 1) * 64, :],
                    )
                    nc.vector.tensor_copy(
                        out=va[b][:, kt, h * VOFF:h * VOFF + D], in_=pst[:]
                    )
                    if pumped:
                        yield

        def pump_gen(work):
            for item in work:
                if item[0] == "vt":
                    yield from vt_chunk(item[1], item[2])
                else:
                    yield from proj_chunk(item[1], item[2], item[3], pumped=True)

        def attend(b, qc, pump=None, rate=2, pending=None, last=False):
            # attn^T accumulators per head: rows 0-63 = dims, row 64 = sum(exp)
            att = [
                ps_att.tile([D + 1, QCHUNK], f32, tag="att", name="attps")
                for _ in range(HPC)
            ]
            exs = [None] * NKT
            qsl = qT[b][:, qc * QCHUNK:(qc + 1) * QCHUNK]

            def att_mms(kt):
                for h in range(HPC):
                    nc.tensor.matmul(
                        att[h][:],
                        lhsT=va[b][:, kt, h * VOFF:h * VOFF + D + 1],
                        rhs=exs[kt][:, h * 512:(h + 1) * 512],
                        start=(kt == 0),
                        stop=(kt == NKT - 1),
                    )

            def att_mm(kt, h):
                nc.tensor.matmul(
                    att[h][:],
                    lhsT=va[b][:, kt, h * VOFF:h * VOFF + D + 1],
                    rhs=exs[kt][:, h * 512:(h + 1) * 512],
                    start=(kt == 0),
                    stop=(kt == NKT - 1),
                )

            DEFER = 4
            for kt in range(NKT):
                # attT deferred by DEFER kts and split around the score pair:
                # [att_h0(kt-D), sc(kt), att_h1(kt-D)] — the next scores never
                # queue behind attT's wait on a fresh Exp, and each attT
                # LDWEIGHTS can background-load under the neighboring matmuls
                if kt >= DEFER:
                    att_mm(kt - DEFER, 0)
                # both heads' scores in one 2-bank PSUM tile; disjoint PE row
                # groups (0/64) let the pair run concurrently
                sc = ps_sc.tile([128, 1024], f32, tag="sc", name="scps")
                for h in range(HPC):
                    nc.tensor.matmul(
                        sc[:, h * 512:(h + 1) * 512],
                        lhsT=kT[b][h * 64:(h + 1) * 64, kt * 128:(kt + 1) * 128],
                        rhs=qsl[h * 64:(h + 1) * 64, :],
                        start=True,
                        stop=True,
                    )
                ex = expp.tile([128, 1024], bf, tag="ex", name="ex")
                nc.scalar.activation(
                    out=ex[:], in_=sc[:], func=mybir.ActivationFunctionType.Exp
                )
                exs[kt] = ex
                if kt >= DEFER:
                    att_mm(kt - DEFER, 1)
                # pump BEFORE the pending drain: the pump's bias-adds gate
                # the next PE matmul group, so they must not queue behind the
                # norm thunks' DVE burst in the strict-FIFO DVE queue
                if pump is not None:
                    for _ in range(rate):
                        next(pump, None)
                if pending:
                    for _ in range(3):
                        if pending:
                            pending.pop(0)()
            def mk_norm(h, att=att, b=b, qc=qc):
                def norm(h=h, att=att, b=b, qc=qc):
                    # reciprocal_approx_fast is bitwise and CANNOT read PSUM
                    # on HW (raw accumulator bits); stage the row via a plain
                    # copy first
                    den = rcp.tile([1, QCHUNK], f32, tag="den", name="den")
                    nc.vector.tensor_copy(out=den[:], in_=att[h][D:D + 1, :])
                    rec = rcp.tile([1, QCHUNK], f32, tag="rec", name="rec")
                    nc.vector.reciprocal_approx_fast(out=rec[:], in_=den[:])
                    rb = rbp.tile([D, QCHUNK], f32, tag="rb", name="rb")
                    nc.gpsimd.partition_broadcast(rb[:], rec[:], channels=D)
                    osb = osp.tile([D, QCHUNK], f32, tag="osb", name="osb")
                    nc.vector.tensor_mul(out=osb[:], in0=att[h][0:D, :], in1=rb[:])
                    nc.sync.dma_start(out=out_d[b, h, :, qc, :], in_=osb[:])
                return norm

            while pending:
                pending.pop(0)()
            if last:
                # tail: h0's (PE-free) normalize starts while h1's last
                # matmul is still on the PE
                for kt in range(NKT - DEFER, NKT - 1):
                    att_mm(kt, 0)
                    att_mm(kt, 1)
                att_mm(NKT - 1, 0)
                mk_norm(0)()
                att_mm(NKT - 1, 1)
                mk_norm(1)()
                return []
            # defer the last att pairs AND the (PE-free) normalize tail into
            # the next attend's kt loop: its first scores then issue right
            # after our last Exp with nothing queued in front of them
            tail = [
                (lambda kt=kt, h=h: att_mm(kt, h))
                for kt in range(NKT - DEFER, NKT)
                for h in range(HPC)
            ]
            return tail + [mk_norm(0), mk_norm(1)]

        # head: project (b0, t0) inline as soon as chunk 0 lands; v0's
        # transposes go to the pump so attend(0,0)'s first scores don't sit
        # behind them in the PE queue
        for which in (1, 0, 2):  # k, q, v
            for _ in proj_chunk(0, 0, which, pumped=False, vt=False, head=True):
                pass

        # everything else is pumped into the attention loops, in emission-
        # deadline order (see per-attend rate math in the build history)
        P = lambda b, t, w: ("p", b, t, w)
        work = (
            [("vt", 0, 0),
             P(0, 1, 1), P(0, 1, 2), P(0, 2, 1), P(0, 2, 2),
             P(0, 3, 1), P(0, 3, 2), P(0, 1, 0), P(0, 2, 0), P(0, 3, 0)]
            + [P(1, 0, 1), P(1, 0, 0), P(1, 0, 2),
               P(1, 1, 1), P(1, 1, 2), P(1, 1, 0),
               P(1, 2, 1), P(1, 2, 2), P(1, 2, 0),
               P(1, 3, 1), P(1, 3, 2), P(1, 3, 0)]
        )
        pump = pump_gen(work)

        pend = attend(0, 0, pump, rate=6)
        for qc in range(1, NQC):
            pend = attend(0, qc, pump, rate=2, pending=pend)
        for qc in range(NQC):
            pend = attend(
                1, qc, pump, rate=2, pending=pend, last=(qc == NQC - 1)
            )
        for _ in pump:
            pass

    nc.compile()
    return nc


def _get_nc():
    if "nc" not in _BUILT:
        _BUILT["nc"] = _build_bass()
    return _BUILT["nc"]


def _prep_core_inputs(x_ch, W, b):
    """Per-core input dicts. W/b slicing+scaling+casting is host-side weight prep."""
    _id64 = np.concatenate([np.eye(64), np.eye(64)], axis=0).astype(_BF16)
    in_maps = []
    for c in range(NCORES):
        q0 = 2 * c * D          # first col of this core's head pair
        wq = W[:, q0:q0 + 128] * SCALING
        wk = W[:, E + q0:E + q0 + 128]
        wv = W[:, 2 * E + q0:2 * E + q0 + 128]
        # [E, 384] -> [KT, 128, 384] -> [128, KT, 384]
        wqkv = np.ascontiguousarray(
            np.concatenate([wq, wk, wv], axis=1)
            .reshape(KT, 128, 384)
            .transpose(1, 0, 2)
        ).astype(_BF16)
        bqkv = np.concatenate(
            [b[q0:q0 + 128] * SCALING, b[E + q0:E + q0 + 128],
             b[2 * E + q0:2 * E + q0 + 128]]
        ).astype(np.float32)[:, None]
        in_maps.append(
            {
                "xch": x_ch,
                "wqkv": wqkv,
                "bqkv": np.ascontiguousarray(bqkv),
                "id64": _id64,
            }
        )
    return in_maps


def _unshard_out(core_outs):
    """[NCORES][B, HPC, D, NQC, QCHUNK] f32 -> [S, B, E] (pure layout)."""
    full = np.empty((S, B, E), np.float32)
    for c, co in enumerate(core_outs):
        # [B, HPC, D, NQC, QCHUNK] -> [NQC, QCHUNK, B, HPC, D] -> [S, B, 128]
        full[:, :, c * 128:(c + 1) * 128] = (
            np.asarray(co).transpose(3, 4, 0, 1, 2).reshape(S, B, 2 * D)
        )
    return full


def run(inputs, trace=False):
    """Returns (output [S,B,E] fp32, BassKernelResults)."""
    from concourse.bass_utils import run_bass_kernel_spmd

    x = np.asarray(inputs["x"], np.float32)
    W = np.asarray(inputs["W_in"], np.float32)
    b = np.asarray(inputs["b_in"], np.float32)
    # sharding prep: cast + lay x out token-chunk-major [B, NQC, 128, KT, 512]
    x_ch = np.ascontiguousarray(
        x.reshape(NQC, QCHUNK, B, KT, 128).transpose(2, 0, 4, 3, 1)
    ).astype(_BF16)

    nc = _get_nc()
    in_maps = _prep_core_inputs(x_ch, W, b)
    res = run_bass_kernel_spmd(
        nc, in_maps, core_ids=list(range(NCORES)), trace=trace
    )
    out = _unshard_out([r["out"] for r in res.results])
    return out, res


def kernel(**inputs):
    out, _ = run(inputs, trace=False)
    return out


# revision 45
# speedup vs baseline: 1.0324x; 1.0324x over previous
"""Multi-head attention (S=2048, B=2, E=1024, H=16, D=64) on 8 Trainium2 cores.

Sharding: batch*heads head-parallel. Core c owns heads {2c, 2c+1} for both
batch elements (4 of the 32 (b,h) attention pairs). Host-side prep: slice/
scale/cast in_proj weights per core, cast x to bf16 laid out token-chunk-major
as [B, NQC, 128, KT, 512] so projection of q-chunk t needs only 1/8th of x;
the output is written head-major [B, 2, 64, NQC, 512] and reassembled to
[S, B, E] on the host (pure layout, no FLOPs). Per core:
  1. ALL inputs stream on the sync DMA queue as ONE need-ordered sequence
     (id64, bias, weights, b0 t0..t3, b1 t0..t3): desc-gens run at kernel
     start on the idle sync sequencer, the scalar queue stays exclusive to
     Exp dispatch, and (b0,t0) projection starts as soon as chunk 0 lands
     (~16us). A PE warm-up burst on id64 beats the HAM cold clock. All other
     projection work is pumped into the attention loops in emission-deadline
     order (generator yields one PE op per pump slot; rates 6/2/2... per
     attend keep every consumer emitted before its producer-read).
  2. q,k,v projected transposed ([col, tok], col = 2 heads x 64) with fp32
     PSUM accumulation over 8 E-tiles; bias added during DVE evacuation
     (q/bias pre-scaled by D^-0.5); head chunks borrow the idle sc PSUM pool.
     v is PE-transposed per 128-kpos tile into va [128, kt, 160] =
     [v_h0 |1| pad | v_h1@80 |1| pad] (ones from one memset = softmax
     denominators; 16-aligned offsets).
  3. Attention per (b, q-chunk of 512): per kpos tile both heads' score MMs
     fill one [128,1024] PSUM tile concurrently (disjoint PE row groups);
     one ScalarE Exp evacuates to SBUF bf16 (the engine floor: 128x ~1.06us).
     attT accumulation (va.T @ exp, row 64 = sum(exp)) is DEFERRED BY FOUR
     kt iterations, split around the score pair [att_h0(kt-4), sc(kt),
     att_h1(kt-4)], and each attend's last 4 att pairs + normalize are
     carried as thunks into the NEXT attend's kt loop (drained 3/kt): the
     Exp stream crosses attend boundaries gaplessly (b1 phase measures
     back-to-back 1004ns Exps).
  4. Normalize without PE transposes: DVE copy of the PSUM denominator row
     (reciprocal_approx_fast is bitwise and misreads PSUM on HW!) -> DVE
     reciprocal_approx_fast -> gpsimd partition_broadcast to [64,512] ->
     DVE multiply (att PSUM x rb SBUF) -> DMA out on sync.
Measured on trn2: 207.6us HW exec; rel err 3.8e-3.
Previous-session baseline (output PE-transposed + per-qchunk normalize
matmul-transposes): 225.8us.
"""

import numpy as np
import ml_dtypes

S, B, E = 2048, 2, 1024
H, D = 16, 64
SCALING = D ** -0.5
NCORES = 8
SB = S * B
HPC = H // NCORES     # 2 heads per core
KT = E // 128         # 8 contraction tiles over E
QCHUNK = 512
NQC = S // QCHUNK     # 4 q/token chunks
NKT = S // 128        # 16 kpos tiles
# va cols: [v_h0(64) | 1 | pad | v_h1(64)@80 | 1 | pad] — head offsets are
# 16-element aligned (XBAR transpose-DMA writes 16-element chunks; unaligned
# destinations corrupt neighbors on HW), and VN itself is 16-aligned so every
# kt slot stays aligned.
VN = 160
VOFF = 80             # head 1 column offset

_BF16 = ml_dtypes.bfloat16
_BUILT = {}


def _build_bass():
    import concourse.bacc as bacc
    import concourse.mybir as mybir
    import concourse.tile as tile
    from contextlib import ExitStack

    f32 = mybir.dt.float32
    bf = mybir.dt.bfloat16

    nc = bacc.Bacc(None, target_bir_lowering=False, debug=False)

    xch_in = nc.dram_tensor("xch", [B, NQC, 128, KT, QCHUNK], bf, kind="ExternalInput")
    wqkv_in = nc.dram_tensor("wqkv", [128, KT, 384], bf, kind="ExternalInput")
    bqkv_in = nc.dram_tensor("bqkv", [384, 1], f32, kind="ExternalInput")
    id64_in = nc.dram_tensor("id64", [128, 64], bf, kind="ExternalInput")
    out_d = nc.dram_tensor("out", [B, HPC, D, NQC, QCHUNK], f32, kind="ExternalOutput")

    with tile.TileContext(nc) as tc, ExitStack() as ctx:
        const = ctx.enter_context(tc.tile_pool(name="const", bufs=1))
        res = ctx.enter_context(tc.tile_pool(name="res", bufs=1))
        expp = ctx.enter_context(tc.tile_pool(name="expp", bufs=8))
        rcp = ctx.enter_context(tc.tile_pool(name="rcp", bufs=4))
        rbp = ctx.enter_context(tc.tile_pool(name="rbp", bufs=4))
        osp = ctx.enter_context(tc.tile_pool(name="osp", bufs=4))
        ps_sc = ctx.enter_context(tc.tile_pool(name="ps_sc", bufs=2, space="PSUM"))
        ps_att = ctx.enter_context(tc.tile_pool(name="ps_att", bufs=2, space="PSUM"))
        ps_pj = ctx.enter_context(tc.tile_pool(name="ps_pj", bufs=1, space="PSUM"))
        ps_vt = ctx.enter_context(tc.tile_pool(name="ps_vt", bufs=1, space="PSUM"))

        # ---- ALL inputs on the sync queue as ONE need-ordered stream: the
        # data lands in exactly this order at full aggregate DMA bandwidth,
        # the desc-gens all run at kernel start on the (otherwise idle) sync
        # sequencer, and the scalar queue stays exclusive to Exp dispatch.
        id64 = const.tile([128, D], bf, tag="id64")
        nc.sync.dma_start(out=id64[:], in_=id64_in[:])
        bqkv_sb = const.tile([128, 3], f32, tag="bqkv")
        nc.sync.dma_start(
            out=bqkv_sb[:], in_=bqkv_in.rearrange("(c p) o -> p (c o)", p=128)
        )
        wq_all = const.tile([128, KT, 384], bf, tag="wqkv")
        nc.sync.dma_start(out=wq_all[:], in_=wqkv_in[:])

        xch = [
            [res.tile([128, KT, QCHUNK], bf, tag=f"xch{b}_{t}", name=f"xch{b}_{t}")
             for t in range(NQC)]
            for b in range(B)
        ]
        for b in range(B):
            for t in range(NQC):
                nc.sync.dma_start(out=xch[b][t][:], in_=xch_in[b, t])

        qT = [res.tile([128, S], bf, tag=f"qT{b}", name=f"qTt{b}") for b in range(B)]
        kT = [res.tile([128, S], bf, tag=f"kT{b}", name=f"kTt{b}") for b in range(B)]
        vT = [res.tile([128, S], bf, tag=f"vT{b}", name=f"vTt{b}") for b in range(B)]
        va = [res.tile([128, NKT, VN], bf, tag=f"va{b}", name=f"vat{b}") for b in range(B)]
        for b in range(B):
            nc.vector.memset(va[b][:], 1.0)  # ones cols survive at 64, 129

        # PE warm-up burst on id64 (lands in ~1us, long before wqkv/x) while
        # the x stream is still in flight (HAM cold clock)
        warm = ps_pj.tile([128, QCHUNK], f32, tag="pj", name="warm")
        for _ in range(30):
            nc.tensor.matmul(
                warm[0:D, 0:D], lhsT=id64[0:D, :], rhs=id64[0:D, :],
                start=True, stop=True,
            )

        def proj_chunk(b, t, which, pumped, vt=True, head=False):
            # out[col, tok] for col-chunk `which` (0=q, 1=k, 2=v), tokens of
            # chunk t. Generator when pumped: yields after each PE op.
            # Head chunks borrow the (still idle) sc pool so the three proj
            # groups don't serialize on the single ps_pj slot.
            dst = (qT[b], kT[b], vT[b])[which]
            if head:
                ps = ps_sc.tile([128, 1024], f32, tag="sc", name="hproj")[:, 0:QCHUNK]
            else:
                ps = ps_pj.tile([128, QCHUNK], f32, tag="pj", name="projps")
            for k in range(KT):
                nc.tensor.matmul(
                    ps[:],
                    lhsT=wq_all[:, k, which * 128:(which + 1) * 128],
                    rhs=xch[b][t][:, k, :],
                    start=(k == 0),
                    stop=(k == KT - 1),
                )
                if k < KT - 1:
                    if pumped:
                        yield
                    continue
                nc.vector.tensor_scalar_add(
                    out=dst[:, t * QCHUNK:(t + 1) * QCHUNK],
                    in0=ps[:],
                    scalar1=bqkv_sb[:, which:which + 1],
                )
                if pumped:
                    yield
            if which == 2 and vt:
                yield from vt_chunk(b, t, pumped)

        def vt_chunk(b, t, pumped=True):
            # PE transpose + DVE evacuation into va. (Both XBAR-transpose-DMA
            # variants measured SLOWER: ~1.2us desc-gen per op serializes on
            # whichever sequencer issues it — even for b1's slack-rich va.)
            for kt in range(4 * t, 4 * t + 4):
                for h in range(HPC):
                    pst = ps_vt.tile([128, D], bf, tag="vt", name="vtps")
                    nc.tensor.transpose(
                        pst[:],
                        in_=vT[b][h * 64:(h + 1) * 64, kt * 128:(kt + 1) * 128],
                        identity=id64[h * 64:(h + 1) * 64, :],
                    )
                    nc.vector.tensor_copy(
                        out=va[b][:, kt, h * VOFF:h * VOFF + D], in_=pst[:]
                    )
                    if pumped:
                        yield

        def pump_gen(work):
            for item in work:
                if item[0] == "vt":
                    yield from vt_chunk(item[1], item[2])
                else:
                    yield from proj_chunk(item[1], item[2], item[3], pumped=True)

        def attend(b, qc, pump=None, rate=2, pending=None, last=False):
            # attn^T accumulators per head: rows 0-63 = dims, row 64 = sum(exp)
            att = [
                ps_att.tile([D + 1, QCHUNK], f32, tag="att", name="attps")
                for _ in range(HPC)
            ]
            exs = [None] * NKT
            qsl = qT[b][:, qc * QCHUNK:(qc + 1) * QCHUNK]

            def att_mms(kt):
                for h in range(HPC):
                    nc.tensor.matmul(
                        att[h][:],
                        lhsT=va[b][:, kt, h * VOFF:h * VOFF + D + 1],
                        rhs=exs[kt][:, h * 512:(h + 1) * 512],
                        start=(kt == 0),
                        stop=(kt == NKT - 1),
                    )

            def att_mm(kt, h):
                nc.tensor.matmul(
                    att[h][:],
                    lhsT=va[b][:, kt, h * VOFF:h * VOFF + D + 1],
                    rhs=exs[kt][:, h * 512:(h + 1) * 512],
                    start=(kt == 0),
                    stop=(kt == NKT - 1),
                )

            DEFER = 4
            for kt in range(NKT):
                # attT deferred by DEFER kts and split around the score pair:
                # [att_h0(kt-D), sc(kt), att_h1(kt-D)] — the next scores never
                # queue behind attT's wait on a fresh Exp, and each attT
                # LDWEIGHTS can background-load under the neighboring matmuls
                if kt >= DEFER:
                    att_mm(kt - DEFER, 0)
                # both heads' scores in one 2-bank PSUM tile; disjoint PE row
                # groups (0/64) let the pair run concurrently
                sc = ps_sc.tile([128, 1024], f32, tag="sc", name="scps")
                for h in range(HPC):
                    nc.tensor.matmul(
                        sc[:, h * 512:(h + 1) * 512],
                        lhsT=kT[b][h * 64:(h + 1) * 64, kt * 128:(kt + 1) * 128],
                        rhs=qsl[h * 64:(h + 1) * 64, :],
                        start=True,
                        stop=True,
                    )
                ex = expp.tile([128, 1024], bf, tag="ex", name="ex")
                nc.scalar.activation(
                    out=ex[:], in_=sc[:], func=mybir.ActivationFunctionType.Exp
                )
                exs[kt] = ex
                if kt >= DEFER:
                    att_mm(kt - DEFER, 1)
                if pending:
                    for _ in range(3):
                        if pending:
                            pending.pop(0)()
                if pump is not None:
                    for _ in range(rate):
                        next(pump, None)
            def mk_norm(h, att=att, b=b, qc=qc):
                def norm(h=h, att=att, b=b, qc=qc):
                    # reciprocal_approx_fast is bitwise and CANNOT read PSUM
                    # on HW (raw accumulator bits); stage the row via a plain
                    # copy first
                    den = rcp.tile([1, QCHUNK], f32, tag="den", name="den")
                    nc.vector.tensor_copy(out=den[:], in_=att[h][D:D + 1, :])
                    rec = rcp.tile([1, QCHUNK], f32, tag="rec", name="rec")
                    nc.vector.reciprocal_approx_fast(out=rec[:], in_=den[:])
                    rb = rbp.tile([D, QCHUNK], f32, tag="rb", name="rb")
                    nc.gpsimd.partition_broadcast(rb[:], rec[:], channels=D)
                    osb = osp.tile([D, QCHUNK], f32, tag="osb", name="osb")
                    nc.vector.tensor_mul(out=osb[:], in0=att[h][0:D, :], in1=rb[:])
                    nc.sync.dma_start(out=out_d[b, h, :, qc, :], in_=osb[:])
                return norm

            while pending:
                pending.pop(0)()
            if last:
                # tail: h0's (PE-free) normalize starts while h1's last
                # matmul is still on the PE
                for kt in range(NKT - DEFER, NKT - 1):
                    att_mm(kt, 0)
                    att_mm(kt, 1)
                att_mm(NKT - 1, 0)
                mk_norm(0)()
                att_mm(NKT - 1, 1)
                mk_norm(1)()
                return []
            # defer the last att pairs AND the (PE-free) normalize tail into
            # the next attend's kt loop: its first scores then issue right
            # after our last Exp with nothing queued in front of them.
            # Head-major order: norm0 launches as soon as h0's accumulation
            # stops, overlapping its DVE chain with h1's remaining matmuls.
            tail = []
            for h in range(HPC):
                tail += [
                    (lambda kt=kt, h=h: att_mm(kt, h))
                    for kt in range(NKT - DEFER, NKT)
                ]
                tail.append(mk_norm(h))
            return tail

        # head: project (b0, t0) inline as soon as chunk 0 lands; v0's
        # transposes go to the pump so attend(0,0)'s first scores don't sit
        # behind them in the PE queue
        for which in (1, 0, 2):  # k, q, v
            for _ in proj_chunk(0, 0, which, pumped=False, vt=False, head=True):
                pass

        # everything else is pumped into the attention loops, in emission-
        # deadline order (see per-attend rate math in the build history)
        P = lambda b, t, w: ("p", b, t, w)
        work = (
            [("vt", 0, 0),
             P(0, 1, 1), P(0, 1, 2), P(0, 2, 1), P(0, 2, 2),
             P(0, 3, 1), P(0, 3, 2), P(0, 1, 0), P(0, 2, 0), P(0, 3, 0)]
            + [P(1, 0, 1), P(1, 0, 0), P(1, 0, 2),
               P(1, 1, 1), P(1, 1, 2), P(1, 1, 0),
               P(1, 2, 1), P(1, 2, 2), P(1, 2, 0),
               P(1, 3, 1), P(1, 3, 2), P(1, 3, 0)]
        )
        pump = pump_gen(work)

        pend = attend(0, 0, pump, rate=6)
        for qc in range(1, NQC):
            pend = attend(0, qc, pump, rate=2, pending=pend)
        for qc in range(NQC):
            pend = attend(
                1, qc, pump, rate=2, pending=pend, last=(qc == NQC - 1)
            )
        for _ in pump:
            pass

    nc.compile()
    return nc


def _get_nc():
    if "nc" not in _BUILT:
        _BUILT["nc"] = _build_bass()
    return _BUILT["nc"]


def _prep_core_inputs(x_ch, W, b):
    """Per-core input dicts. W/b slicing+scaling+casting is host-side weight prep."""
    _id64 = np.concatenate([np.eye(64), np.eye(64)], axis=0).astype(_BF16)
    in_maps = []
    for c in range(NCORES):
        q0 = 2 * c * D          # first col of this core's head pair
        wq = W[:, q0:q0 + 128] * SCALING
        wk = W[:, E + q0:E + q0 + 128]
        wv = W[:, 2 * E + q0:2 * E + q0 + 128]
        # [E, 384] -> [KT, 128, 384] -> [128, KT, 384]
        wqkv = np.ascontiguousarray(
            np.concatenate([wq, wk, wv], axis=1)
            .reshape(KT, 128, 384)
            .transpose(1, 0, 2)
        ).astype(_BF16)
        bqkv = np.concatenate(
            [b[q0:q0 + 128] * SCALING, b[E + q0:E + q0 + 128],
             b[2 * E + q0:2 * E + q0 + 128]]
        ).astype(np.float32)[:, None]
        in_maps.append(
            {
                "xch": x_ch,
                "wqkv": wqkv,
                "bqkv": np.ascontiguousarray(bqkv),
                "id64": _id64,
            }
        )
    return in_maps


def _unshard_out(core_outs):
    """[NCORES][B, HPC, D, NQC, QCHUNK] f32 -> [S, B, E] (pure layout)."""
    full = np.empty((S, B, E), np.float32)
    for c, co in enumerate(core_outs):
        # [B, HPC, D, NQC, QCHUNK] -> [NQC, QCHUNK, B, HPC, D] -> [S, B, 128]
        full[:, :, c * 128:(c + 1) * 128] = (
            np.asarray(co).transpose(3, 4, 0, 1, 2).reshape(S, B, 2 * D)
        )
    return full


def run(inputs, trace=False):
    """Returns (output [S,B,E] fp32, BassKernelResults)."""
    from concourse.bass_utils import run_bass_kernel_spmd

    x = np.asarray(inputs["x"], np.float32)
    W = np.asarray(inputs["W_in"], np.float32)
    b = np.asarray(inputs["b_in"], np.float32)
    # sharding prep: cast + lay x out token-chunk-major [B, NQC, 128, KT, 512]
    x_ch = np.ascontiguousarray(
        x.reshape(NQC, QCHUNK, B, KT, 128).transpose(2, 0, 4, 3, 1)
    ).astype(_BF16)

    nc = _get_nc()
    in_maps = _prep_core_inputs(x_ch, W, b)
    res = run_bass_kernel_spmd(
        nc, in_maps, core_ids=list(range(NCORES)), trace=trace
    )
    out = _unshard_out([r["out"] for r in res.results])
    return out, res


def kernel(**inputs):
    out, _ = run(inputs, trace=False)
    return out
